# revision 23
# baseline (speedup 1.0000x reference)
"""Fused LayerNorm + multi-head attention + output projection on 8 TRN2 NeuronCores.

Sharding: 2-way data parallel over batch x 4-way tensor parallel over heads.
Core c handles batch (c // 4), heads [4*(c%4) .. 4*(c%4)+4).

Device dataflow (everything transposed: host supplies x^T so the feature/
contraction dim always lands on SBUF partitions):
  - LayerNorm is folded into the QKV-projection epilogue:
      qkv^T[n,i] = rstd_i * (raw[n,i] - mu_i * wsum_n) (+ wb_n)
    with raw = W'^T x^T computed on raw x, row stats (mu, rstd) from
    PE ones-matmuls (which broadcast across partitions for free).
  - Scores are computed transposed (S^T[j,i]) so softmax'd probs feed the
    PV matmul without any transpose; two 64-dim heads are packed into the
    128 PE rows via tile_position row groups (they execute concurrently).
  - Softmax skips max-subtraction (values are bounded; a constant bias in
    the exp cancels in the normalization). The denominator comes from an
    extra ones-column appended to V (M=65 PV matmul).
  - The attention inner loop is software-pipelined for the in-order engine
    queues: each iteration emits scores(j+1) first (keeping the ACT exp
    stream fed), then one drip unit of background work (stats / qkv groups
    / v transposes / output projection), then PV(j) (which must wait for
    exp(j) anyway - the drip work fills that gap). This keeps the PE dense,
    which also keeps its DVFS p-state at max clock.
  - Output projection for i-chunk c is emitted as drip work inside chunk
    c+1's attention loop; the host sums the 4 TP partials per batch, adds
    b_out, and transposes back.
"""

import os
import sys
from collections import deque

import numpy as np

for _p in ("/root/.axon_site", "/root/.axon_site/_ro/trn_rl_repo",
           "/root/.axon_site/_ro/pypackages", "/opt/trn_rl_repo"):
    if os.path.isdir(_p) and _p not in sys.path:
        sys.path.append(_p)

B = 2
N = 2048
DIM = 1024
HEADS = 16
DIM_HEAD = 64
INNER = HEADS * DIM_HEAD
HEADS_PER_CORE = 4          # 4-way tensor parallel on heads
N_CORES = 8
EPS = 1e-5
EXP_BIAS = -4.0             # constant subtracted inside exp; cancels in softmax

KT = DIM // 128             # 8 k-tiles of the contraction dim
IC = 4                      # i-chunks of 512 over N=2048
ICW = N // IC               # 512
JT = N // 128               # 16 j-tiles
NQKV = 3 * HEADS_PER_CORE * DIM_HEAD   # 768 local qkv columns
NT = NQKV // 128            # 6 n-tiles: [q01, q23, k01, k23, v01, v23]
MT = DIM // 128             # 8 output m-tiles

_COMPILED = {}


def _build(has_wb):
    import concourse.bass as bass
    import concourse.mybir as mybir
    from concourse import bacc, tile
    from concourse.masks import make_identity
    from contextlib import ExitStack

    f32 = mybir.dt.float32
    bf16 = mybir.dt.bfloat16
    AF = mybir.ActivationFunctionType
    ALU = mybir.AluOpType

    nc = bacc.Bacc("TRN2", target_bir_lowering=False, debug=False,
                   num_devices=N_CORES)

    xT_d = nc.dram_tensor("xT", [DIM, N], bf16, kind="ExternalInput")
    wqkv_d = nc.dram_tensor("wqkv", [DIM, NQKV], bf16, kind="ExternalInput")
    wsum_d = nc.dram_tensor("wsum", [NQKV, 1], f32, kind="ExternalInput")
    wout_d = nc.dram_tensor("wout", [HEADS_PER_CORE * DIM_HEAD, DIM], bf16,
                            kind="ExternalInput")
    wb_d = nc.dram_tensor("wb", [NQKV, 1], f32, kind="ExternalInput")
    out_d = nc.dram_tensor("outT", [DIM, N], bf16, kind="ExternalOutput")

    with ExitStack() as ctx:
        tc = ctx.enter_context(tile.TileContext(nc))
        cst = ctx.enter_context(tc.tile_pool(name="cst", bufs=1))
        xp = ctx.enter_context(tc.tile_pool(name="xp", bufs=KT))
        wp = ctx.enter_context(tc.tile_pool(name="wp", bufs=KT))
        qkp = ctx.enter_context(tc.tile_pool(name="qk", bufs=1))
        vtp = ctx.enter_context(tc.tile_pool(name="vt", bufs=1))
        vaugp = ctx.enter_context(tc.tile_pool(name="vaug", bufs=JT))
        bcp = ctx.enter_context(tc.tile_pool(name="bc", bufs=1))
        scp = ctx.enter_context(tc.tile_pool(name="sc", bufs=2))
        ep = ctx.enter_context(tc.tile_pool(name="ep", bufs=6))
        onp = ctx.enter_context(tc.tile_pool(name="on", bufs=2 * IC))
        otp = ctx.enter_context(tc.tile_pool(name="ot", bufs=3))
        smp = ctx.enter_context(tc.tile_pool(name="sm", bufs=2))
        # single PSUM pool, 8 banks total:
        #   tag "s": 2 x [128,1024] (2 banks each) -> 4 banks (score tiles)
        #   tag "o": 2 x [128,512]                 -> 2 banks (PV accums)
        #   tag "g": 2 x [128,512]                 -> 2 banks
        #            (LN stats, qkv groups, v transposes, out projection)
        psum = ctx.enter_context(tc.tile_pool(name="psum", bufs=2,
                                              space="PSUM"))

        # ---- constants ----
        ones = cst.tile([128, 128], bf16)
        nc.vector.memset(ones[:], 1.0)
        ebias_t = cst.tile([128, 1], f32, tag="ebias")
        nc.vector.memset(ebias_t[:], EXP_BIAS)
        magic_t = cst.tile([128, ICW], mybir.dt.int32, tag="magic")
        nc.vector.memset(magic_t[:], 0x5F3759DF)
        ident = cst.tile([128, 128], bf16)
        make_identity(nc, ident[:])
        wb_t = cst.tile([128, NT], f32)

        # ---- input DMAs, in need-order ----
        # x chunk loads split in half so two queues carry each k-tile.
        xt = [xp.tile([128, N], bf16, tag="xt", name=f"xt{k}")
              for k in range(KT)]
        wt = [wp.tile([128, NQKV], bf16, tag="wt", name=f"wt{k}")
              for k in range(KT)]
        wsum_t = cst.tile([128, NT], f32)

        def load_x_chunk(c):
            for k in range(KT):
                for h in range(2):
                    sl = slice(c * ICW + h * (ICW // 2),
                               c * ICW + (h + 1) * (ICW // 2))
                    nc.sync.dma_start(xt[k][:, sl],
                                      xT_d[k * 128:(k + 1) * 128, sl])

        def load_w_group(nt):
            nsl = slice(nt * 128, (nt + 1) * 128)
            for k in range(KT):
                nc.sync.dma_start(wt[k][:, nsl],
                                  wqkv_d[k * 128:(k + 1) * 128, nsl])
            nc.sync.dma_start(wsum_t[:, nt:nt + 1],
                              wsum_d[nt * 128:(nt + 1) * 128, :])
            if has_wb:
                nc.sync.dma_start(wb_t[:, nt:nt + 1],
                                  wb_d[nt * 128:(nt + 1) * 128, :])

        load_x_chunk(0)
        for nt in (0, 2, 4):
            load_w_group(nt)
        for c in range(1, IC):
            load_x_chunk(c)
        for nt in (1, 3, 5):
            load_w_group(nt)
        wo = []
        for d in range(2):
            t = cst.tile([128, DIM], bf16, tag=f"wo{d}", name=f"wo{d}")
            for h in range(2):
                sl = slice(h * (DIM // 2), (h + 1) * (DIM // 2))
                nc.sync.dma_start(t[:, sl], wout_d[d * 128:(d + 1) * 128, sl])
            wo.append(t)

        # ---- persistent activation tiles ----
        mu_bc = bcp.tile([128, N], f32, tag="mu")        # holds -mu
        rstd_bc = bcp.tile([128, N], f32, tag="rstd")    # holds +rstd
        q01 = qkp.tile([128, N], bf16, tag="q01")
        q23 = qkp.tile([128, N], bf16, tag="q23")
        k01 = qkp.tile([128, N], bf16, tag="k01")
        k23 = qkp.tile([128, N], bf16, tag="k23")
        vT = [vtp.tile([128, N], bf16, tag=f"vt{i}", name=f"vt{i}")
              for i in range(2)]
        qkv_dst = [q01, q23, k01, k23, vT[0], vT[1]]
        vaug = [[None] * JT for _ in range(2)]

        # ---- demand-driven emission ----
        emitted = set()

        def ensure(key, fn, *args):
            if key not in emitted:
                emitted.add(key)
                fn(*args)

        # stats for one i-chunk: mu (as -mu) and rstd columns.
        # Kept atomic so the two "g" PSUM slots it holds are released by
        # DVE ops queued immediately after - any interleaved "g" alloc
        # would stall the PE on readers that aren't emitted yet.
        def stats_emit(ic):
            isl = slice(ic * ICW, (ic + 1) * ICW)
            sum_ps = psum.tile([128, ICW], f32, tag="g", name="sum_ps")
            sq_ps = psum.tile([128, ICW], f32, tag="g", name="sq_ps")
            for k in range(KT):
                x2 = scp.tile([128, ICW], bf16, tag="x2", bufs=3, name="x2")
                nc.vector.tensor_mul(x2[:], xt[k][:, isl], xt[k][:, isl])
                nc.tensor.matmul(sum_ps[:], ones[:], xt[k][:, isl],
                                 start=(k == 0), stop=(k == KT - 1))
                nc.tensor.matmul(sq_ps[:], ones[:], x2[:],
                                 start=(k == 0), stop=(k == KT - 1))
            nc.vector.tensor_scalar_mul(mu_bc[:, isl], sum_ps[:], -1.0 / DIM)
            msq = scp.tile([128, ICW], f32, tag="msq", bufs=1, name="msq")
            nc.vector.tensor_scalar(msq[:], sq_ps[:], 1.0 / DIM, EPS,
                                    op0=ALU.mult, op1=ALU.add)
            mu2 = scp.tile([128, ICW], f32, tag="mu2", bufs=1, name="mu2")
            nc.vector.tensor_mul(mu2[:], mu_bc[:, isl], mu_bc[:, isl])
            var = scp.tile([128, ICW], f32, tag="var", bufs=1, name="var")
            nc.vector.tensor_sub(var[:], msq[:], mu2[:])
            if ic == 0:
                # before the exp stream starts: short ACT chain (the Exp
                # table stays loaded for the whole attention stream)
                lnv = scp.tile([128, ICW], f32, tag="lnv", bufs=1, name="lnv")
                nc.scalar.activation(lnv[:], var[:], AF.Ln)
                nc.scalar.activation(rstd_bc[:, isl], lnv[:], AF.Exp,
                                     scale=-0.5)
                return
            # streamed stats: 1/sqrt(var) via bit-trick seed + 2 Newton
            # steps, all on DVE (an ACT Ln here would force an activation
            # table reload in the middle of the exp stream)
            y0 = scp.tile([128, ICW], f32, tag="y0", bufs=1, name="y0")
            half_i = y0.bitcast(mybir.dt.int32)
            nc.vector.tensor_scalar(half_i[:], var.bitcast(mybir.dt.int32)[:],
                                    1, None, op0=ALU.arith_shift_right)
            nc.vector.scalar_tensor_tensor(half_i[:], magic_t[:], 1,
                                           half_i[:], op0=ALU.bypass,
                                           op1=ALU.subtract)
            t1 = scp.tile([128, ICW], f32, tag="t1", bufs=1, name="t1")
            nc.vector.tensor_mul(t1[:], y0[:], y0[:])
            nc.vector.tensor_mul(t1[:], t1[:], var[:])
            nc.vector.tensor_scalar(t1[:], t1[:], -0.5, 1.5,
                                    op0=ALU.mult, op1=ALU.add)
            nc.vector.tensor_mul(y0[:], y0[:], t1[:])
            nc.vector.tensor_mul(t1[:], y0[:], y0[:])
            nc.vector.tensor_mul(t1[:], t1[:], var[:])
            nc.vector.tensor_scalar(t1[:], t1[:], -0.5, 1.5,
                                    op0=ALU.mult, op1=ALU.add)
            nc.vector.tensor_mul(rstd_bc[:, isl], y0[:], t1[:])

        def ensure_stats(ic):
            ensure(("st", ic), stats_emit, ic)

        def qkv_emit(nt, ic):
            ensure_stats(ic)
            nsl = slice(nt * 128, (nt + 1) * 128)
            isl = slice(ic * ICW, (ic + 1) * ICW)
            ps = psum.tile([128, ICW], f32, tag="g", name="qkv_ps")
            for k in range(KT):
                nc.tensor.matmul(ps[:], wt[k][:, nsl], xt[k][:, isl],
                                 start=(k == 0), stop=(k == KT - 1))
            # (raw - mu*wsum) * rstd [+ wb]; the epilogue lives on the DVE
            # so the PE matmul chain never depends on the LN stats chain.
            tmp = scp.tile([128, ICW], f32, tag="fix", bufs=3, name="fix")
            nc.vector.scalar_tensor_tensor(
                tmp[:], mu_bc[:, isl], wsum_t[:, nt:nt + 1], ps[:],
                op0=ALU.mult, op1=ALU.add)
            dst = qkv_dst[nt][:, isl]
            nc.vector.tensor_mul(dst, tmp[:], rstd_bc[:, isl])
            if has_wb:
                nc.vector.tensor_scalar_add(dst, dst, wb_t[:, nt:nt + 1])

        def tp_emit(d, j):
            """v^T -> v_aug[d][j]: [v_h | 1] blocks for the two local heads."""
            ensure(("g", 4 + d, j // 4), qkv_emit, 4 + d, j // 4)
            va = vaugp.tile([128, 2 * 65], bf16, tag=f"vaug{d}",
                            name=f"vaug{d}_{j}", bufs=JT)
            vaug[d][j] = va
            nc.vector.memset(va[:, 64:2 * 65:65], 1.0)
            tp = psum.tile([128, 128], bf16, tag="g", name="tp")
            nc.tensor.transpose(tp[:], vT[d][:, j * 128:(j + 1) * 128],
                                ident[:])
            nc.vector.tensor_copy(va[:, 0:64], tp[:, 0:64])
            nc.vector.tensor_copy(va[:, 65:129], tp[:, 64:128])

        def ensure_qkv(nt, ic):
            ensure(("g", nt, ic), qkv_emit, nt, ic)

        def ensure_tp(d, j):
            ensure(("tp", d, j), tp_emit, d, j)

        def op_emit(ic, mt):
            """Output projection for one (i-chunk, m-tile)."""
            isl = slice(ic * ICW, (ic + 1) * ICW)
            msl = slice(mt * 128, (mt + 1) * 128)
            pps = psum.tile([128, ICW], f32, tag="g", name="pj_ps")
            for d in range(2):
                nc.tensor.matmul(pps[:], wo[d][:, msl], o_norm[d][ic][:],
                                 start=(d == 0), stop=(d == 1))
            ot = otp.tile([128, ICW], bf16, tag="ot", name="ot")
            nc.vector.tensor_copy(ot[:], pps[:])
            for h in range(2):
                csl = slice(h * (ICW // 2), (h + 1) * (ICW // 2))
                nc.sync.dma_start(
                    out_d[msl, ic * ICW + h * (ICW // 2):
                          ic * ICW + (h + 1) * (ICW // 2)], ot[:, csl])

        # drip schedule: background work consumed one unit per attention
        # iteration, placed between scores(j+1) and PV(j) so it fills the
        # exp-stream wait without ever delaying the ACT engine. Only the
        # work with deadlines inside pair0's window is pre-listed; pair1's
        # own k/v/transpose groups are emitted by the JIT ensures in its
        # attention iterations, which land in the same drip slot position.
        drip = deque()
        for c in (1, 2, 3):
            drip.append(("st", c))
            drip.append(("g", 2, c))        # k01 chunk c
            drip.append(("g", 4, c))        # v01 chunk c
            for j in range(4 * c, 4 * c + 4):
                drip.append(("tp", 0, j))
            drip.append(("g", 0, c))        # q01 chunk c (next ic prologue)
        drip.append(("g", 1, 0))            # q23/k23 chunk 0 so the
        drip.append(("g", 3, 0))            # pair boundary crosses smoothly

        def drip_one():
            while drip:
                key = drip.popleft()
                if key in emitted:
                    continue
                if key[0] == "g":
                    ensure_qkv(key[1], key[2])
                elif key[0] == "st":
                    ensure(key, stats_emit, key[1])
                elif key[0] == "tp":
                    ensure_tp(key[1], key[2])
                elif key[0] == "op":
                    op_emit(key[1], key[2])
                return

        # ---- attention + output projection ----
        qt_pair = [q01, q23]
        kt_pair = [k01, k23]
        o_norm = [[onp.tile([128, ICW], bf16, tag="onorm",
                            name=f"onorm{p}_{i}") for i in range(IC)]
                  for p in range(2)]

        def emit_scores(pair, ic, j):
            """scores + exp for j-tile j; returns the prob tile."""
            isl = slice(ic * ICW, (ic + 1) * ICW)
            jsl = slice(j * 128, (j + 1) * 128)
            kt = kt_pair[pair]
            qt = qt_pair[pair]
            s_ps = psum.tile([128, 2 * ICW], f32, tag="s", name="s_ps")
            e_t = ep.tile([128, 2 * ICW], bf16, tag="e", name="e_t")
            for hh in range(2):
                psl = slice(hh * 64, (hh + 1) * 64)
                nc.tensor.matmul(s_ps[:, hh * ICW:(hh + 1) * ICW],
                                 kt[psl, jsl], qt[psl, isl])
            nc.scalar.activation(e_t[:], s_ps[:], AF.Exp,
                                 bias=ebias_t[:, 0:1])
            return e_t

        def normalize(pair, ic, o_ps):
            """O[d,i] / l_i  (l is row 64 of o_ps)."""
            # DVE copy out of PSUM first: frees the two "o" banks within
            # ~1.2us so the next i-chunk's PV accumulation isn't stalled
            # behind this chain (which runs ~4us deep otherwise).
            o_sb = [smp.tile([65, ICW], f32, tag="osb", name=f"osb{hh}",
                             bufs=4) for hh in range(2)]
            lrow = [smp.tile([1, ICW], f32, tag="lrow", name=f"lrow{hh}",
                             bufs=4) for hh in range(2)]
            linv = [smp.tile([1, ICW], f32, tag="linv", name=f"linv{hh}",
                             bufs=4) for hh in range(2)]
            lbc = [smp.tile([64, ICW], f32, tag="lbc", name=f"lbc{hh}",
                            bufs=4) for hh in range(2)]
            for hh in range(2):
                nc.vector.tensor_copy(o_sb[hh][:], o_ps[hh][0:65, :])
            for hh in range(2):
                nc.sync.dma_start(lrow[hh][:], o_sb[hh][64:65, :])
            for hh in range(2):
                nc.vector.reciprocal_approx_fast(linv[hh][:], lrow[hh][:])
            for hh in range(2):
                nc.gpsimd.partition_broadcast(lbc[hh][:], linv[hh][:])
            nc.vector.tensor_mul(o_norm[pair][ic][0:64, :],
                                 o_sb[0][0:64, :], lbc[0][:])
            ob = smp.tile([64, ICW], bf16, tag="ob", name="ob", bufs=2)
            nc.vector.tensor_mul(ob[:], o_sb[1][0:64, :], lbc[1][:])
            for h in range(2):
                csl = slice(h * (ICW // 2), (h + 1) * (ICW // 2))
                nc.sync.dma_start(o_norm[pair][ic][64:128, csl], ob[:, csl])

        # one flat software pipeline over all (pair, i-chunk, j-tile)
        # iterations; the scores/exp for iteration t+1 are emitted during
        # iteration t, including across i-chunk and pair boundaries, so the
        # ACT exp stream never waits at a boundary.
        seq = [(pair, ic, j) for pair in range(2) for ic in range(IC)
               for j in range(JT)]
        ensure_qkv(0, 0)
        ensure_qkv(2, 0)
        e_pend = emit_scores(0, 0, 0)
        o_ps = None
        for t, (pair, ic, j) in enumerate(seq):
            if j == 0:
                o_ps = [psum.tile([128, ICW], f32, tag="o", name="o_ps")
                        for _ in range(2)]
            e_cur = e_pend
            if t + 1 < len(seq):
                npair, nic, nj = seq[t + 1]
                ensure_qkv(npair, nic)
                ensure_qkv(2 + npair, nj // 4)
                e_pend = emit_scores(npair, nic, nj)
            drip_one()
            ensure_tp(pair, j)
            for hh in range(2):
                nc.tensor.matmul(
                    o_ps[hh][0:65, :],
                    vaug[pair][j][:, 65 * hh:65 * hh + 65],
                    e_cur[:, hh * ICW:(hh + 1) * ICW],
                    start=(j == 0), stop=(j == JT - 1))
            if j == JT - 1:
                normalize(pair, ic, o_ps)
                if pair == 1:
                    if ic < IC - 1:
                        for mt in range(MT):
                            drip.append(("op", ic, mt))
                    else:
                        while drip:
                            drip_one()
                        for mt in range(MT):
                            op_emit(ic, mt)

    nc.compile()
    return nc


def _get_compiled(has_wb):
    key = bool(has_wb)
    if key not in _COMPILED:
        _COMPILED[key] = _build(key)
    return _COMPILED[key]


def _shard_inputs(x, ln_gamma, ln_beta, w_qkv, w_out):
    """Build per-core input maps (all host-side layout work, no math on x)."""
    import ml_dtypes
    bf = ml_dtypes.bfloat16

    x = np.ascontiguousarray(np.asarray(x, np.float32))
    g = np.asarray(ln_gamma, np.float32)
    be = np.asarray(ln_beta, np.float32)
    wq = np.asarray(w_qkv, np.float32)
    wo = np.asarray(w_out, np.float32)

    scale = DIM_HEAD ** (-0.5)
    wq_g = wq * g[:, None]            # gamma folded
    wq_g[:, :INNER] *= scale          # attention scale folded into W_q
    wb_full = be @ wq                 # beta contribution
    wb_full[:INNER] *= scale

    in_maps = []
    for c in range(N_CORES):
        b = c // HEADS_PER_CORE
        grp = c % HEADS_PER_CORE
        heads = [HEADS_PER_CORE * grp + t for t in range(HEADS_PER_CORE)]
        # column order: [q01, q23, k01, k23, v01, v23] pair-tiles
        cols = []
        for which in range(3):        # q, k, v
            for h in heads:
                lo = which * INNER + h * DIM_HEAD
                cols.append(np.arange(lo, lo + DIM_HEAD))
        cols = np.concatenate(cols)
        wqkv_c = np.ascontiguousarray(wq_g[:, cols])
        # bf16-round the weights before computing wsum so the LN-fold
        # correction matches what the device matmul actually sums.
        wqkv_bf = wqkv_c.astype(bf)
        wsum_c = wqkv_bf.astype(np.float32).sum(axis=0).reshape(NQKV, 1)
        wb_c = wb_full[cols].reshape(NQKV, 1)
        rows = np.concatenate([np.arange(h * DIM_HEAD, (h + 1) * DIM_HEAD)
                               for h in heads])
        wout_c = np.ascontiguousarray(wo[rows, :])
        in_maps.append({
            "xT": np.ascontiguousarray(x[b].T).astype(bf),
            "wqkv": wqkv_bf,
            "wsum": np.ascontiguousarray(wsum_c),
            "wout": wout_c.astype(bf),
            "wb": np.ascontiguousarray(wb_c),
        })
    return in_maps


def _run(inputs, trace=False):
    from concourse.bass_utils import run_bass_kernel_spmd

    in_maps = _shard_inputs(inputs["x"], inputs["ln_gamma"],
                            inputs["ln_beta"], inputs["w_qkv"],
                            inputs["w_out"])
    has_wb = bool(np.max(np.abs(in_maps[0]["wb"])) > 0)
    nc = _get_compiled(has_wb)
    res = run_bass_kernel_spmd(nc, in_maps, core_ids=list(range(N_CORES)),
                               trace=trace)
    b_out = np.asarray(inputs["b_out"], np.float32)
    outs = []
    for b in range(B):
        acc = np.zeros((DIM, N), np.float32)
        for grp in range(HEADS_PER_CORE):
            acc += res.results[b * HEADS_PER_CORE + grp]["outT"].astype(
                np.float32)
        outs.append(acc.T + b_out)
    out = np.stack(outs).astype(np.float32)
    return out, res


def kernel(**inputs):
    out, _ = _run(inputs, trace=False)
    return out


# revision 24
# speedup vs baseline: 1.0530x; 1.0530x over previous
"""Fused LayerNorm + multi-head attention + output projection on 8 TRN2 NeuronCores.

Sharding: 2-way data parallel over batch x 4-way tensor parallel over heads.
Core c handles batch (c // 4), heads [4*(c%4) .. 4*(c%4)+4).

Device dataflow (everything transposed: host supplies x^T so the feature/
contraction dim always lands on SBUF partitions):
  - LayerNorm is folded into the QKV-projection epilogue:
      qkv^T[n,i] = rstd_i * (raw[n,i] - mu_i * wsum_n) (+ wb_n)
    with raw = W'^T x^T computed on raw x, row stats (mu, rstd) from
    PE ones-matmuls (which broadcast across partitions for free).
  - Scores are computed transposed (S^T[j,i]) so softmax'd probs feed the
    PV matmul without any transpose; two 64-dim heads are packed into the
    128 PE rows via tile_position row groups (they execute concurrently).
  - Softmax skips max-subtraction (values are bounded; a constant bias in
    the exp cancels in the normalization). The denominator comes from an
    extra ones-column appended to V (M=65 PV matmul).
  - The attention inner loop is software-pipelined for the in-order engine
    queues: each iteration emits scores(j+1) first (keeping the ACT exp
    stream fed), then one drip unit of background work (stats / qkv groups
    / v transposes / output projection), then PV(j) (which must wait for
    exp(j) anyway - the drip work fills that gap). This keeps the PE dense,
    which also keeps its DVFS p-state at max clock.
  - Output projection for i-chunk c is emitted as drip work inside chunk
    c+1's attention loop; the host sums the 4 TP partials per batch, adds
    b_out, and transposes back.
"""

import os
import sys
from collections import deque

import numpy as np

for _p in ("/root/.axon_site", "/root/.axon_site/_ro/trn_rl_repo",
           "/root/.axon_site/_ro/pypackages", "/opt/trn_rl_repo"):
    if os.path.isdir(_p) and _p not in sys.path:
        sys.path.append(_p)

B = 2
N = 2048
DIM = 1024
HEADS = 16
DIM_HEAD = 64
INNER = HEADS * DIM_HEAD
HEADS_PER_CORE = 4          # 4-way tensor parallel on heads
N_CORES = 8
EPS = 1e-5
EXP_BIAS = -4.0             # constant subtracted inside exp; cancels in softmax

KT = DIM // 128             # 8 k-tiles of the contraction dim
IC = 4                      # i-chunks of 512 over N=2048
ICW = N // IC               # 512
JT = N // 128               # 16 j-tiles
NQKV = 3 * HEADS_PER_CORE * DIM_HEAD   # 768 local qkv columns
NT = NQKV // 128            # 6 n-tiles: [q01, q23, k01, k23, v01, v23]
MT = DIM // 128             # 8 output m-tiles

_COMPILED = {}


def _build(has_wb):
    import concourse.bass as bass
    import concourse.mybir as mybir
    from concourse import bacc, tile
    from concourse.masks import make_identity
    from contextlib import ExitStack

    f32 = mybir.dt.float32
    bf16 = mybir.dt.bfloat16
    AF = mybir.ActivationFunctionType
    ALU = mybir.AluOpType

    nc = bacc.Bacc("TRN2", target_bir_lowering=False, debug=False,
                   num_devices=N_CORES)

    xT_d = nc.dram_tensor("xT", [DIM, N], bf16, kind="ExternalInput")
    wqkv_d = nc.dram_tensor("wqkv", [DIM, NQKV], bf16, kind="ExternalInput")
    wsum_d = nc.dram_tensor("wsum", [NQKV, 1], f32, kind="ExternalInput")
    wout_d = nc.dram_tensor("wout", [HEADS_PER_CORE * DIM_HEAD, DIM], bf16,
                            kind="ExternalInput")
    wb_d = nc.dram_tensor("wb", [NQKV, 1], f32, kind="ExternalInput")
    out_d = nc.dram_tensor("outT", [DIM, N], bf16, kind="ExternalOutput")

    with ExitStack() as ctx:
        tc = ctx.enter_context(tile.TileContext(nc))
        cst = ctx.enter_context(tc.tile_pool(name="cst", bufs=1))
        xp = ctx.enter_context(tc.tile_pool(name="xp", bufs=KT))
        wp = ctx.enter_context(tc.tile_pool(name="wp", bufs=KT))
        qkp = ctx.enter_context(tc.tile_pool(name="qk", bufs=1))
        vtp = ctx.enter_context(tc.tile_pool(name="vt", bufs=1))
        vaugp = ctx.enter_context(tc.tile_pool(name="vaug", bufs=JT))
        bcp = ctx.enter_context(tc.tile_pool(name="bc", bufs=1))
        scp = ctx.enter_context(tc.tile_pool(name="sc", bufs=2))
        ep = ctx.enter_context(tc.tile_pool(name="ep", bufs=6))
        onp = ctx.enter_context(tc.tile_pool(name="on", bufs=2 * IC))
        otp = ctx.enter_context(tc.tile_pool(name="ot", bufs=3))
        smp = ctx.enter_context(tc.tile_pool(name="sm", bufs=2))
        # single PSUM pool, 8 banks total:
        #   tag "s": 2 x [128,1024] (2 banks each) -> 4 banks (score tiles)
        #   tag "o": 2 x [128,512]                 -> 2 banks (PV accums)
        #   tag "g": 2 x [128,512]                 -> 2 banks
        #            (LN stats, qkv groups, v transposes, out projection)
        psum = ctx.enter_context(tc.tile_pool(name="psum", bufs=2,
                                              space="PSUM"))

        # ---- constants ----
        ones = cst.tile([128, 128], bf16)
        nc.vector.memset(ones[:], 1.0)
        ebias_t = cst.tile([128, 1], f32, tag="ebias")
        nc.vector.memset(ebias_t[:], EXP_BIAS)
        magic_t = cst.tile([128, ICW], mybir.dt.int32, tag="magic")
        nc.vector.memset(magic_t[:], 0x5F3759DF)
        ident = cst.tile([128, 128], bf16)
        make_identity(nc, ident[:])
        wb_t = cst.tile([128, NT], f32)

        # ---- input DMAs, in need-order ----
        # x chunk loads split in half so two queues carry each k-tile.
        xt = [xp.tile([128, N], bf16, tag="xt", name=f"xt{k}")
              for k in range(KT)]
        wt = [wp.tile([128, NQKV], bf16, tag="wt", name=f"wt{k}")
              for k in range(KT)]
        wsum_t = cst.tile([128, NT], f32)

        def load_x_chunk(c):
            for k in range(KT):
                for h in range(2):
                    sl = slice(c * ICW + h * (ICW // 2),
                               c * ICW + (h + 1) * (ICW // 2))
                    nc.sync.dma_start(xt[k][:, sl],
                                      xT_d[k * 128:(k + 1) * 128, sl])

        def load_w_group(nt):
            nsl = slice(nt * 128, (nt + 1) * 128)
            for k in range(KT):
                nc.sync.dma_start(wt[k][:, nsl],
                                  wqkv_d[k * 128:(k + 1) * 128, nsl])
            nc.sync.dma_start(wsum_t[:, nt:nt + 1],
                              wsum_d[nt * 128:(nt + 1) * 128, :])
            if has_wb:
                nc.sync.dma_start(wb_t[:, nt:nt + 1],
                                  wb_d[nt * 128:(nt + 1) * 128, :])

        load_x_chunk(0)
        for nt in (0, 2, 4):
            load_w_group(nt)
        for c in range(1, IC):
            load_x_chunk(c)
        for nt in (1, 3, 5):
            load_w_group(nt)
        wo = []
        for d in range(2):
            t = cst.tile([128, DIM], bf16, tag=f"wo{d}", name=f"wo{d}")
            for h in range(2):
                sl = slice(h * (DIM // 2), (h + 1) * (DIM // 2))
                nc.sync.dma_start(t[:, sl], wout_d[d * 128:(d + 1) * 128, sl])
            wo.append(t)

        # ---- persistent activation tiles ----
        mu_bc = bcp.tile([128, N], f32, tag="mu")        # holds -mu
        rstd_bc = bcp.tile([128, N], f32, tag="rstd")    # holds +rstd
        q01 = qkp.tile([128, N], bf16, tag="q01")
        q23 = qkp.tile([128, N], bf16, tag="q23")
        k01 = qkp.tile([128, N], bf16, tag="k01")
        k23 = qkp.tile([128, N], bf16, tag="k23")
        vT = [vtp.tile([128, N], bf16, tag=f"vt{i}", name=f"vt{i}")
              for i in range(2)]
        qkv_dst = [q01, q23, k01, k23, vT[0], vT[1]]
        vaug = [[None] * JT for _ in range(2)]

        # ---- demand-driven emission ----
        emitted = set()

        def ensure(key, fn, *args):
            if key not in emitted:
                emitted.add(key)
                fn(*args)

        # stats for one i-chunk: mu (as -mu) and rstd columns.
        # Kept atomic so the two "g" PSUM slots it holds are released by
        # DVE ops queued immediately after - any interleaved "g" alloc
        # would stall the PE on readers that aren't emitted yet.
        def stats_emit(ic):
            isl = slice(ic * ICW, (ic + 1) * ICW)
            sum_ps = psum.tile([128, ICW], f32, tag="g", name="sum_ps")
            sq_ps = psum.tile([128, ICW], f32, tag="g", name="sq_ps")
            for k in range(KT):
                x2 = scp.tile([128, ICW], bf16, tag="x2", bufs=3, name="x2")
                nc.vector.tensor_mul(x2[:], xt[k][:, isl], xt[k][:, isl])
                nc.tensor.matmul(sum_ps[:], ones[:], xt[k][:, isl],
                                 start=(k == 0), stop=(k == KT - 1))
                nc.tensor.matmul(sq_ps[:], ones[:], x2[:],
                                 start=(k == 0), stop=(k == KT - 1))
            nc.vector.tensor_scalar_mul(mu_bc[:, isl], sum_ps[:], -1.0 / DIM)
            msq = scp.tile([128, ICW], f32, tag="msq", bufs=1, name="msq")
            nc.vector.tensor_scalar(msq[:], sq_ps[:], 1.0 / DIM, EPS,
                                    op0=ALU.mult, op1=ALU.add)
            mu2 = scp.tile([128, ICW], f32, tag="mu2", bufs=1, name="mu2")
            nc.vector.tensor_mul(mu2[:], mu_bc[:, isl], mu_bc[:, isl])
            var = scp.tile([128, ICW], f32, tag="var", bufs=1, name="var")
            nc.vector.tensor_sub(var[:], msq[:], mu2[:])
            if ic == 0:
                # before the exp stream starts: short ACT chain (the Exp
                # table stays loaded for the whole attention stream)
                lnv = scp.tile([128, ICW], f32, tag="lnv", bufs=1, name="lnv")
                nc.scalar.activation(lnv[:], var[:], AF.Ln)
                nc.scalar.activation(rstd_bc[:, isl], lnv[:], AF.Exp,
                                     scale=-0.5)
                return
            # streamed stats: 1/sqrt(var) via bit-trick seed + 2 Newton
            # steps, all on DVE (an ACT Ln here would force an activation
            # table reload in the middle of the exp stream)
            y0 = scp.tile([128, ICW], f32, tag="y0", bufs=1, name="y0")
            half_i = y0.bitcast(mybir.dt.int32)
            nc.vector.tensor_scalar(half_i[:], var.bitcast(mybir.dt.int32)[:],
                                    1, None, op0=ALU.arith_shift_right)
            nc.vector.scalar_tensor_tensor(half_i[:], magic_t[:], 1,
                                           half_i[:], op0=ALU.bypass,
                                           op1=ALU.subtract)
            t1 = scp.tile([128, ICW], f32, tag="t1", bufs=1, name="t1")
            nc.vector.tensor_mul(t1[:], y0[:], y0[:])
            nc.vector.tensor_mul(t1[:], t1[:], var[:])
            nc.vector.tensor_scalar(t1[:], t1[:], -0.5, 1.5,
                                    op0=ALU.mult, op1=ALU.add)
            nc.vector.tensor_mul(y0[:], y0[:], t1[:])
            nc.vector.tensor_mul(t1[:], y0[:], y0[:])
            nc.vector.tensor_mul(t1[:], t1[:], var[:])
            nc.vector.tensor_scalar(t1[:], t1[:], -0.5, 1.5,
                                    op0=ALU.mult, op1=ALU.add)
            nc.vector.tensor_mul(rstd_bc[:, isl], y0[:], t1[:])

        def ensure_stats(ic):
            ensure(("st", ic), stats_emit, ic)

        def qkv_emit(nt, ic):
            ensure_stats(ic)
            nsl = slice(nt * 128, (nt + 1) * 128)
            isl = slice(ic * ICW, (ic + 1) * ICW)
            ps = psum.tile([128, ICW], f32, tag="g", name="qkv_ps")
            for k in range(KT):
                nc.tensor.matmul(ps[:], wt[k][:, nsl], xt[k][:, isl],
                                 start=(k == 0), stop=(k == KT - 1))
            # (raw - mu*wsum) * rstd [+ wb]; the epilogue lives on the DVE
            # so the PE matmul chain never depends on the LN stats chain.
            tmp = scp.tile([128, ICW], f32, tag="fix", bufs=3, name="fix")
            nc.vector.scalar_tensor_tensor(
                tmp[:], mu_bc[:, isl], wsum_t[:, nt:nt + 1], ps[:],
                op0=ALU.mult, op1=ALU.add)
            dst = qkv_dst[nt][:, isl]
            nc.vector.tensor_mul(dst, tmp[:], rstd_bc[:, isl])
            if has_wb:
                nc.vector.tensor_scalar_add(dst, dst, wb_t[:, nt:nt + 1])

        def tp_emit(d, j):
            """v^T -> v_aug[d][j]: [v_h | 1] blocks for the two local heads."""
            ensure(("g", 4 + d, j // 4), qkv_emit, 4 + d, j // 4)
            va = vaugp.tile([128, 2 * 65], bf16, tag=f"vaug{d}",
                            name=f"vaug{d}_{j}", bufs=JT)
            vaug[d][j] = va
            nc.vector.memset(va[:, 64:2 * 65:65], 1.0)
            tp = psum.tile([128, 128], bf16, tag="g", name="tp")
            nc.tensor.transpose(tp[:], vT[d][:, j * 128:(j + 1) * 128],
                                ident[:])
            nc.vector.tensor_copy(va[:, 0:64], tp[:, 0:64])
            nc.vector.tensor_copy(va[:, 65:129], tp[:, 64:128])

        def ensure_qkv(nt, ic):
            ensure(("g", nt, ic), qkv_emit, nt, ic)

        def ensure_tp(d, j):
            ensure(("tp", d, j), tp_emit, d, j)

        def op_emit(ic, mt):
            """Output projection for one (i-chunk, m-tile)."""
            isl = slice(ic * ICW, (ic + 1) * ICW)
            msl = slice(mt * 128, (mt + 1) * 128)
            pps = psum.tile([128, ICW], f32, tag="g", name="pj_ps")
            for d in range(2):
                nc.tensor.matmul(pps[:], wo[d][:, msl], o_norm[d][ic][:],
                                 start=(d == 0), stop=(d == 1))
            ot = otp.tile([128, ICW], bf16, tag="ot", name="ot")
            nc.vector.tensor_copy(ot[:], pps[:])
            for h in range(2):
                csl = slice(h * (ICW // 2), (h + 1) * (ICW // 2))
                nc.sync.dma_start(
                    out_d[msl, ic * ICW + h * (ICW // 2):
                          ic * ICW + (h + 1) * (ICW // 2)], ot[:, csl])

        # drip schedule: background work consumed one unit per attention
        # iteration, placed between scores(j+1) and PV(j) so it fills the
        # exp-stream wait. CAUTION: a drip unit whose input DMA hasn't
        # landed stalls the whole in-order PE queue (and with it the exp
        # stream), so chunk-c k/v groups are never pre-emitted ahead of
        # chunk c's x arrival - the j-loop's JIT ensures pull them exactly
        # when needed; the drip only lists work that is safely resident.
        drip = deque()
        drip.append(("st", 1))
        drip.append(("g", 0, 1))
        drip.append(("g", 2, 2))
        for j in range(8, 12):
            drip.append(("tp", 0, j))
        drip.append(("st", 2))
        drip.append(("g", 0, 2))
        drip.append(("g", 2, 3))
        for j in range(12, 16):
            drip.append(("tp", 0, j))
        drip.append(("st", 3))
        drip.append(("g", 0, 3))
        drip.append(("g", 1, 0))
        drip.append(("g", 3, 0))            # k23 chunk 0: smooth pair cross
        drip.append(("g", 1, 1))
        drip.append(("g", 1, 2))
        drip.append(("g", 1, 3))

        def drip_one():
            while drip:
                key = drip.popleft()
                if key in emitted:
                    continue
                if key[0] == "g":
                    ensure_qkv(key[1], key[2])
                elif key[0] == "st":
                    ensure(key, stats_emit, key[1])
                elif key[0] == "tp":
                    ensure_tp(key[1], key[2])
                elif key[0] == "op":
                    op_emit(key[1], key[2])
                return

        # ---- attention + output projection ----
        qt_pair = [q01, q23]
        kt_pair = [k01, k23]
        o_norm = [[onp.tile([128, ICW], bf16, tag="onorm",
                            name=f"onorm{p}_{i}") for i in range(IC)]
                  for p in range(2)]

        def emit_scores(pair, ic, j):
            """scores + exp for j-tile j; returns the prob tile."""
            isl = slice(ic * ICW, (ic + 1) * ICW)
            jsl = slice(j * 128, (j + 1) * 128)
            kt = kt_pair[pair]
            qt = qt_pair[pair]
            s_ps = psum.tile([128, 2 * ICW], f32, tag="s", name="s_ps")
            e_t = ep.tile([128, 2 * ICW], bf16, tag="e", name="e_t")
            for hh in range(2):
                psl = slice(hh * 64, (hh + 1) * 64)
                nc.tensor.matmul(s_ps[:, hh * ICW:(hh + 1) * ICW],
                                 kt[psl, jsl], qt[psl, isl])
            nc.scalar.activation(e_t[:], s_ps[:], AF.Exp,
                                 bias=ebias_t[:, 0:1])
            return e_t

        def normalize(pair, ic, o_ps):
            """O[d,i] / l_i  (l is row 64 of o_ps)."""
            # DVE copy out of PSUM first: frees the two "o" banks within
            # ~1.2us so the next i-chunk's PV accumulation isn't stalled
            # behind this chain (which runs ~4us deep otherwise).
            o_sb = [smp.tile([65, ICW], f32, tag="osb", name=f"osb{hh}",
                             bufs=4) for hh in range(2)]
            lrow = [smp.tile([1, ICW], f32, tag="lrow", name=f"lrow{hh}",
                             bufs=4) for hh in range(2)]
            linv = [smp.tile([1, ICW], f32, tag="linv", name=f"linv{hh}",
                             bufs=4) for hh in range(2)]
            lbc = [smp.tile([64, ICW], f32, tag="lbc", name=f"lbc{hh}",
                            bufs=4) for hh in range(2)]
            for hh in range(2):
                nc.vector.tensor_copy(o_sb[hh][:], o_ps[hh][0:65, :])
            for hh in range(2):
                nc.sync.dma_start(lrow[hh][:], o_sb[hh][64:65, :])
            for hh in range(2):
                nc.vector.reciprocal_approx_fast(linv[hh][:], lrow[hh][:])
            for hh in range(2):
                nc.gpsimd.partition_broadcast(lbc[hh][:], linv[hh][:])
            nc.vector.tensor_mul(o_norm[pair][ic][0:64, :],
                                 o_sb[0][0:64, :], lbc[0][:])
            ob = smp.tile([64, ICW], bf16, tag="ob", name="ob", bufs=2)
            nc.vector.tensor_mul(ob[:], o_sb[1][0:64, :], lbc[1][:])
            for h in range(2):
                csl = slice(h * (ICW // 2), (h + 1) * (ICW // 2))
                nc.sync.dma_start(o_norm[pair][ic][64:128, csl], ob[:, csl])

        # one flat software pipeline over all (pair, i-chunk, j-tile)
        # iterations; the scores/exp for iteration t+1 are emitted during
        # iteration t, including across i-chunk and pair boundaries, so the
        # ACT exp stream never waits at a boundary.
        seq = [(pair, ic, j) for pair in range(2) for ic in range(IC)
               for j in range(JT)]
        ensure_qkv(0, 0)
        ensure_qkv(2, 0)
        e_pend = emit_scores(0, 0, 0)
        o_ps = None
        for t, (pair, ic, j) in enumerate(seq):
            if j == 0:
                o_ps = [psum.tile([128, ICW], f32, tag="o", name="o_ps")
                        for _ in range(2)]
            e_cur = e_pend
            if t + 1 < len(seq):
                npair, nic, nj = seq[t + 1]
                ensure_qkv(npair, nic)
                ensure_qkv(2 + npair, nj // 4)
                e_pend = emit_scores(npair, nic, nj)
            drip_one()
            ensure_tp(pair, j)
            for hh in range(2):
                nc.tensor.matmul(
                    o_ps[hh][0:65, :],
                    vaug[pair][j][:, 65 * hh:65 * hh + 65],
                    e_cur[:, hh * ICW:(hh + 1) * ICW],
                    start=(j == 0), stop=(j == JT - 1))
            if j == JT - 1:
                normalize(pair, ic, o_ps)
                if pair == 1:
                    if ic < IC - 1:
                        for mt in range(MT):
                            drip.append(("op", ic, mt))
                    else:
                        while drip:
                            drip_one()
                        for mt in range(MT):
                            op_emit(ic, mt)

    nc.compile()
    return nc


def _get_compiled(has_wb):
    key = bool(has_wb)
    if key not in _COMPILED:
        _COMPILED[key] = _build(key)
    return _COMPILED[key]


def _shard_inputs(x, ln_gamma, ln_beta, w_qkv, w_out):
    """Build per-core input maps (all host-side layout work, no math on x)."""
    import ml_dtypes
    bf = ml_dtypes.bfloat16

    x = np.ascontiguousarray(np.asarray(x, np.float32))
    g = np.asarray(ln_gamma, np.float32)
    be = np.asarray(ln_beta, np.float32)
    wq = np.asarray(w_qkv, np.float32)
    wo = np.asarray(w_out, np.float32)

    scale = DIM_HEAD ** (-0.5)
    wq_g = wq * g[:, None]            # gamma folded
    wq_g[:, :INNER] *= scale          # attention scale folded into W_q
    wb_full = be @ wq                 # beta contribution
    wb_full[:INNER] *= scale

    in_maps = []
    for c in range(N_CORES):
        b = c // HEADS_PER_CORE
        grp = c % HEADS_PER_CORE
        heads = [HEADS_PER_CORE * grp + t for t in range(HEADS_PER_CORE)]
        # column order: [q01, q23, k01, k23, v01, v23] pair-tiles
        cols = []
        for which in range(3):        # q, k, v
            for h in heads:
                lo = which * INNER + h * DIM_HEAD
                cols.append(np.arange(lo, lo + DIM_HEAD))
        cols = np.concatenate(cols)
        wqkv_c = np.ascontiguousarray(wq_g[:, cols])
        # bf16-round the weights before computing wsum so the LN-fold
        # correction matches what the device matmul actually sums.
        wqkv_bf = wqkv_c.astype(bf)
        wsum_c = wqkv_bf.astype(np.float32).sum(axis=0).reshape(NQKV, 1)
        wb_c = wb_full[cols].reshape(NQKV, 1)
        rows = np.concatenate([np.arange(h * DIM_HEAD, (h + 1) * DIM_HEAD)
                               for h in heads])
        wout_c = np.ascontiguousarray(wo[rows, :])
        in_maps.append({
            "xT": np.ascontiguousarray(x[b].T).astype(bf),
            "wqkv": wqkv_bf,
            "wsum": np.ascontiguousarray(wsum_c),
            "wout": wout_c.astype(bf),
            "wb": np.ascontiguousarray(wb_c),
        })
    return in_maps


def _run(inputs, trace=False):
    from concourse.bass_utils import run_bass_kernel_spmd

    in_maps = _shard_inputs(inputs["x"], inputs["ln_gamma"],
                            inputs["ln_beta"], inputs["w_qkv"],
                            inputs["w_out"])
    has_wb = bool(np.max(np.abs(in_maps[0]["wb"])) > 0)
    nc = _get_compiled(has_wb)
    res = run_bass_kernel_spmd(nc, in_maps, core_ids=list(range(N_CORES)),
                               trace=trace)
    b_out = np.asarray(inputs["b_out"], np.float32)
    outs = []
    for b in range(B):
        acc = np.zeros((DIM, N), np.float32)
        for grp in range(HEADS_PER_CORE):
            acc += res.results[b * HEADS_PER_CORE + grp]["outT"].astype(
                np.float32)
        outs.append(acc.T + b_out)
    out = np.stack(outs).astype(np.float32)
    return out, res


def kernel(**inputs):
    out, _ = _run(inputs, trace=False)
    return out
